# revision 30
# baseline (speedup 1.0000x reference)
"""Causal attention (B=8, T=2048, D=1024, fp32 in/out) on 8 trn2 NeuronCores.

Sharding: data-parallel over batch — core b computes batch element b.

S^T formulation: instead of S = Q@K^T (q on partitions) + PE transposes of
P for the PV matmul, compute S^T = K@Q^T directly (k on partitions, q on
free dim).  exp(S^T) then lands in SBUF already in weight orientation for
O = P^T.T @ V — no PE transposes, no PSUM->SBUF copies, and row-sums come
from N=1 matmuls against a ones column (accumulated in PSUM across k).

Two-pass structure per 512-wide q-mega (PSUM is only 8 banks):
  pass1: for kc: S^T tile [128k, 512q] (8 MMs, weights = K chunk),
         exp -> pt_all, rowsums (4 x N=1 MMs)   [banks: st 2, rs 2]
  pass2: per q-block: both PV halves streamed kc 0..qb from stored
         pt_all as solid same-bank chains        [banks: ph0 2, ph1 2]
Block-major pass-2 chains keep every PV matmul on one PSUM bank (216ns
each); interleaving blocks per kc paid ~21ns/matmul in bank switches.
Megas run in descending order so the ramp is compute-bound against the
K^T piece stream, and the final mega's terminal chains are the shortest.
"""

import sys

if "/opt/trn_rl_repo" not in sys.path:
    sys.path.insert(0, "/opt/trn_rl_repo")

import numpy as np

B, T, D = 8, 2048, 1024
NQ = T // 128    # 16 q blocks of 128
NKC = T // 128   # 16 k chunks of 128
ND = D // 128    # 8 d chunks of 128
MEGA = 512       # q-mega width (4 blocks)
NMEGA = T // MEGA
NEG = -1e10
SOFTMAX_SCALE = 1.0 / float(np.sqrt(D))

_CACHE = {}


def _split_waits(nc):
    """This container's walrus accepts only ONE sync-wait per instruction
    (setupSyncWait: 'Too many sync wait commands').  Tile freely attaches
    several waits to one instruction.  Hoist the extras onto same-engine
    NoOps inserted immediately before the instruction — each engine
    executes its stream in order, so the wait semantics are unchanged."""
    import concourse.mybir as mybir

    n_split = 0
    for f in nc.m.functions:
        for bb in f.blocks:
            out = []
            for inst in bb.instructions:
                si = inst.sync_info
                if si is not None and len(si.on_wait) > 1:
                    waits = list(si.on_wait)
                    for w in waits[:-1]:
                        nop = mybir.InstNoOp(
                            name=f"{inst.name}-w{n_split}",
                            engine=inst.engine,
                            sync_info=mybir.SyncInfo(on_wait=[w], on_update=[]),
                            bass_nofuse=True,
                        )
                        out.append(nop)
                        n_split += 1
                    inst.sync_info = mybir.SyncInfo(
                        on_wait=[waits[-1]], on_update=list(si.on_update)
                    )
                out.append(inst)
            bb.instructions[:] = out
    return n_split


def _build():
    import concourse.bass as bass
    import concourse.mybir as mybir
    import concourse.tile as tile

    f32 = mybir.dt.float32
    bf16 = mybir.dt.bfloat16
    EXP = mybir.ActivationFunctionType.Exp

    nc = bass.Bass()
    # Host-swizzled layouts (contiguous per-partition runs on both sides):
    #   Q^T: [p, pass, dc, c]     pass = 512 q-cols (one mega)
    #   K^T: [p, sl, cg, dc, c]   sl = 512 k-cols, cg = 128-col group, so
    #        one (sl, cg) piece is exactly one k-chunk: 2KB/partition,
    #        letting the first slice stream in at kc granularity.
    #   V:   [p, kc, c]
    qt_d = nc.dram_tensor("query_t", [128, NMEGA, ND, MEGA], bf16,
                          kind="ExternalInput")
    kt_d = nc.dram_tensor("key_t", [128, 4, 4, ND, 128], bf16,
                          kind="ExternalInput")
    v_d = nc.dram_tensor("value", [128, NKC, D], bf16, kind="ExternalInput")
    o_d = nc.dram_tensor("out", [T, D], bf16, kind="ExternalOutput")

    with tile.TileContext(nc) as tc:
        with (
            tc.tile_pool(name="const", bufs=1) as constp,
            tc.tile_pool(name="big", bufs=1) as bigp,
            tc.tile_pool(name="qtpool", bufs=2) as qtpool,
            tc.tile_pool(name="ptpool", bufs=2) as ptpool,
            tc.tile_pool(name="osb", bufs=2) as osbp,
            tc.tile_pool(name="small", bufs=4) as smallp,
            tc.tile_pool(name="psum_st", bufs=2, space="PSUM") as psum_st,
            tc.tile_pool(name="psum_o", bufs=2, space="PSUM") as psum_o,
            tc.tile_pool(name="psum_rs", bufs=2, space="PSUM") as psum_rs,
        ):
            # HAM heater source memset first on gpsimd (earliest preamble).
            # The heater borrows an o-tagged PSUM bank (first real PV0 use is
            # ~10us later), freeing a bank so rs can double-buffer across
            # megas.
            heat_src = constp.tile([128, 128], bf16)
            nc.gpsimd.memset(heat_src[:], 1.0)
            # 19 heaters bridge the PE until the first input data nears; the
            # few cold-rate first matmuls overlap the DMA-bound ramp anyway,
            # and longer heater chains just delay the real start 1:1 (A/B
            # measured: 19 heaters 142.6us vs 36-44 heaters 143.6-145.1us).
            heat_ps = psum_o.tile([128, 512], f32, tag="ph0", name="heat")
            for _ in range(19):
                nc.tensor.matmul(heat_ps[:, :128], heat_src[:], heat_src[:],
                                 start=True, stop=True)

            # ones column for rowsum matmuls
            ones = constp.tile([128, 1], bf16)
            nc.gpsimd.memset(ones[:], 1.0)

            # Causal additive mask for a diagonal 128x128 block in S^T
            # orientation: cmask[k_p, q_c] = 0 if q_c >= k_p else NEG.
            cmask = constp.tile([128, 128], f32)
            nc.gpsimd.memset(cmask[:], 0.0)
            nc.gpsimd.affine_select(
                out=cmask[:],
                in_=cmask[:],
                compare_op=mybir.AluOpType.is_ge,
                fill=NEG,
                base=0,
                channel_multiplier=-1,
                pattern=[[1, 128]],
            )

            v_all = bigp.tile([128, NKC, D], bf16)
            kt_all = bigp.tile([128, 4, 4, ND, 128], bf16)

            # ---- DMA issue helpers ---------------------------------------
            qt_tiles = {}

            def issue_qt(m, split=0):
                if m < NMEGA and m not in qt_tiles:
                    qt = qtpool.tile([128, ND, MEGA], bf16, tag="qtpass",
                                     name=f"qtp{m}")
                    if split:
                        step = ND // split
                        for lo in range(0, ND, step):
                            nc.sync.dma_start(qt[:, lo:lo + step],
                                              qt_d[:, m, lo:lo + step])
                    else:
                        nc.sync.dma_start(qt[:], qt_d[:, m])
                    qt_tiles[m] = qt

            def issue_kt_piece(sl, g):
                nc.scalar.dma_start(kt_all[:, sl, g], kt_d[:, sl, g])

            def issue_kt_slice(sl):
                nc.scalar.dma_start(kt_all[:, sl], kt_d[:, sl])

            def issue_v(lo, hi, eng=None):
                (eng or nc.sync).dma_start(v_all[:, lo:hi, :], v_d[:, lo:hi, :])

            def kt_w(kc):  # lhsT [128, 128] for k-chunk kc
                return kt_all[:, kc // 4, kc % 4, :, :]

            # ---- per-mega emitters ---------------------------------------
            st_state = {}

            def emit_rs_one(m, kc, j):
                """One rowsum matmul (N=1) for (chunk kc, block j);
                reciprocal when the block's accumulation chain stops."""
                pt = st_state["pt"]
                rs = st_state["rs"]
                jlo = max(0, kc - 4 * m)
                qb = 4 * m + j
                # start=True marks the ENTIRE 2KB PSUM bank pending-zero,
                # so only the very first rowsum matmul of the mega may
                # carry it — the four column chains share one bank, and
                # each column's first (start=False) write still lands as
                # an overwrite via the per-byte pending-zero bits.
                nc.tensor.matmul(
                    rs[:, j:j + 1],
                    pt[:, kc, j * 128:(j + 1) * 128],
                    ones[:],
                    start=(kc == 0 and j == jlo),
                    stop=(kc == qb),
                    skip_group_check=True,
                )
                if kc == qb:
                    rinv = smallp.tile([128, 1], f32, tag=f"rinv{j}",
                                       name=f"rinv{m}_{j}")
                    nc.vector.reciprocal(rinv[:], rs[:, j:j + 1])
                    st_state["rinv"][j] = rinv

            def emit_rs(m, kc):
                for j in range(max(0, kc - 4 * m), 4):
                    emit_rs_one(m, kc, j)

            def emit_st(m, kc):
                """S^T tile for (kc, mega m): [128 k, 512 q] + exp -> pt.
                (Interleaving the previous chunk's rowsum matmuls inside
                this group was tried and REGRESSED: they wait on that
                chunk's exp, which is still in flight mid-group, stalling
                the in-order PE stream.)"""
                qt = qt_tiles[m]
                jlo = max(0, kc - 4 * m)
                off = 128 * jlo
                stp = psum_st.tile([128, MEGA], f32, tag="st")
                kw = kt_w(kc)
                for dc in range(ND):
                    nc.tensor.matmul(
                        stp[:, off:MEGA],
                        kw[:, dc, :],
                        qt[:, dc, off:MEGA],
                        start=(dc == 0),
                        stop=(dc == ND - 1),
                    )
                if kc >= 4 * m:
                    # diagonal chunk: strict lower triangle (k > q) masked
                    nc.vector.tensor_add(
                        stp[:, off:off + 128], stp[:, off:off + 128], cmask[:]
                    )
                pt = st_state["pt"]
                nc.scalar.activation(
                    pt[:, kc, off:MEGA], stp[:, off:MEGA], EXP,
                    bias=0.0, scale=SOFTMAX_SCALE,
                )

            def emit_pv_block(m, j, store_eng):
                """Both PV halves for q-block j of mega m, streamed kc
                0..qb from stored pt.  Block-major chains keep every matmul
                on the SAME PSUM bank — interleaving the four blocks per kc
                (the old pass-1 PV) paid ~21ns/matmul in bank switches.
                Half0's normalize (DVE) runs in the shadow of half1's
                matmul chain; half1's normalize rides the idle ACT engine."""
                pt = st_state["pt"]
                qb = 4 * m + j
                o0 = psum_o.tile([128, 512], f32, tag="ph0",
                                 name=f"o0_{m}_{j}")
                for kc in range(qb + 1):
                    nc.tensor.matmul(
                        o0[:],
                        pt[:, kc, j * 128:(j + 1) * 128],
                        v_all[:, kc, 0:512],
                        start=(kc == 0),
                        stop=(kc == qb),
                    )
                rinv = st_state["rinv"][j]
                o_sb = osbp.tile([128, D], bf16, tag=f"osb{j}",
                                 name=f"osb{m}_{j}")
                nc.vector.tensor_scalar_mul(o_sb[:, 0:512], o0[:], rinv[:])
                if m == 0:
                    # last mega: ship half0 now so the terminal store moves
                    # only 512 columns
                    nc.sync.dma_start(
                        o_d[qb * 128:(qb + 1) * 128, 0:512], o_sb[:, 0:512])
                o1 = psum_o.tile([128, 512], f32, tag="ph1",
                                 name=f"o1_{m}_{j}")
                for kc in range(qb + 1):
                    nc.tensor.matmul(
                        o1[:],
                        pt[:, kc, j * 128:(j + 1) * 128],
                        v_all[:, kc, 512:1024],
                        start=(kc == 0),
                        stop=(kc == qb),
                    )
                if m == 0 and j == 3:
                    # terminal block: split the normalize across ACT/DVE
                    # and the store across both HWDGE queues so the tail
                    # chain (copy -> descriptor-gen -> wire) runs in two
                    # parallel halves instead of one serial 512-col pass
                    nc.scalar.activation(
                        o_sb[:, 512:768], o1[:, 0:256],
                        mybir.ActivationFunctionType.Copy,
                        bias=0.0, scale=rinv[:],
                    )
                    nc.vector.tensor_scalar_mul(
                        o_sb[:, 768:1024], o1[:, 256:512], rinv[:])
                    nc.scalar.dma_start(
                        o_d[qb * 128:(qb + 1) * 128, 512:768],
                        o_sb[:, 512:768])
                    nc.sync.dma_start(
                        o_d[qb * 128:(qb + 1) * 128, 768:1024],
                        o_sb[:, 768:1024])
                else:
                    nc.scalar.activation(
                        o_sb[:, 512:1024], o1[:],
                        mybir.ActivationFunctionType.Copy,
                        bias=0.0, scale=rinv[:],
                    )
                    lo = 512 if m == 0 else 0
                    store_eng.dma_start(
                        o_d[qb * 128:(qb + 1) * 128, lo:1024],
                        o_sb[:, lo:1024])

            # ---- schedule -------------------------------------------------
            # Megas are processed in DESCENDING order: mega 3 consumes a
            # full 512-wide S^T tile (1.7us of PE work) per 256KB K^T piece,
            # so the ramp is compute-bound as the K stream lands (ascending
            # order idles ~5us waiting for data during the tiny mega 0).
            # Mega 0 last also makes the terminal pass-2 chain the shortest.
            # HWDGE rings drain FIFO per ring, so issue order = arrival
            # order: K^T pieces arrive in exactly the kc consumption order.
            # V rides the sync ring BEHIND Q^T: per-ring FIFO means the v
            # chunks never steal wire from the critical first Q^T/K^T, yet
            # arrive just in time for PV0 (v0 needed ~1.5us after the first
            # S^T completes).  9th+ HWDGE D2Ds stall their sequencer until
            # a completion frees a semaphore lane — ordered so only the
            # non-urgent tail (v2-3, kt1) waits.
            # first k-chunk split in half so the very first S^T matmul's
            # weights (dc 0-3) land one transfer-quantum earlier
            nc.scalar.dma_start(kt_all[:, 0, 0, 0:4], kt_d[:, 0, 0, 0:4])
            nc.scalar.dma_start(kt_all[:, 0, 0, 4:8], kt_d[:, 0, 0, 4:8])
            issue_kt_piece(0, 1)
            issue_qt(3, split=4)
            issue_kt_piece(0, 2)
            issue_v(0, 1)
            # Prime the exp activation table load here: it is inserted right
            # before the first InstActivation on each path, so a dummy
            # activation after the critical first K^T pieces puts the 1.3us
            # table load in the DMA shadow instead of before the first exp.
            act_prime = constp.tile([128, 1], f32)
            nc.scalar.activation(act_prime[:], heat_src[:, 0:1], EXP,
                                 bias=0.0, scale=1.0)
            issue_kt_piece(0, 3)
            issue_v(1, 2)
            issue_v(2, 4)
            issue_kt_slice(1)

            for m in reversed(range(NMEGA)):
                K = 4 * m + 4
                st_state["pt"] = ptpool.tile([128, NKC, MEGA], bf16,
                                             tag="pt", name=f"pt{m}")
                st_state["rs"] = psum_rs.tile([128, 4], f32, tag="rs",
                                              name=f"rs{m}")
                st_state["rinv"] = [None] * 4

                # pass 1: S^T + exp + rowsums, exp latency hidden by
                # deferring rs(kc) until after S^T(kc+1) is emitted.
                for kc in range(K):
                    # staggered prefetches (v0-3 ride the gpsimd SWDGE
                    # queue: separate descriptor generator AND separate
                    # completion-semaphore lanes, so the 8 shared HWDGE
                    # lanes stay with Q^T/K^T/V-batches/stores)
                    if m == 3 and kc == 1:
                        issue_v(4, 8)
                    elif m == 3 and kc == 3:
                        issue_kt_slice(2)
                    elif m == 3 and kc == 5:
                        issue_v(8, 12)
                    elif m == 3 and kc == 7:
                        issue_kt_slice(3)
                    elif m == 3 and kc == 9:
                        issue_v(12, 16)
                    elif m == 3 and kc == 11:
                        issue_qt(2)
                    elif m == 2 and kc == 2:
                        issue_qt(1)
                    elif m == 2 and kc == 6:
                        issue_qt(0)
                    emit_st(m, kc)
                    if kc > 0:
                        emit_rs(m, kc - 1)

                # pass 2 (block-major): block 0's chains stop at kc=4m, so
                # they don't touch the last exp — emit them first to fill
                # the PE while exp(K-1) is in flight, then close pass 1.
                emit_pv_block(m, 0, nc.sync)
                emit_rs(m, K - 1)
                for j in range(1, 4):
                    # last processed mega is m=0: its final stores ride the
                    # scalar queue (idle by then) to overlap the sync ring
                    store = nc.scalar if (m == 0 and j >= 2) else nc.sync
                    emit_pv_block(m, j, store)

    _split_waits(nc)
    return nc


def _np_reference(query, key, value, mask):
    """Host fallback for the general (non-all-ones) padding-mask case."""
    out = np.empty_like(query)
    tri = np.triu(np.ones((T, T), dtype=np.float32), 1) * 1e10
    for b in range(B):
        s = query[b] @ key[b].T
        s = s - tri
        s = s - (1.0 - mask[b])[None, :] * 1e10
        s = s * SOFTMAX_SCALE
        s = s - s.max(axis=-1, keepdims=True)
        p = np.exp(s)
        p = p / p.sum(axis=-1, keepdims=True)
        out[b] = p @ value[b]
    return out


def make_in_maps(query, key, value):
    """Per-core input dicts: batch b -> core b, host-swizzled (see _build)."""
    import ml_dtypes

    bf = ml_dtypes.bfloat16

    def qswizzle(x):  # [T, D] -> [p, pass, dc, c]
        return np.ascontiguousarray(
            x.reshape(NMEGA, MEGA, ND, 128).transpose(3, 0, 2, 1)
        ).astype(bf)

    def kswizzle(x):  # [T, D] -> [p, sl, cg, dc, c]
        return np.ascontiguousarray(
            x.reshape(4, 4, 128, ND, 128).transpose(4, 0, 1, 3, 2)
        ).astype(bf)

    def vswizzle(x):  # [T, D] -> [p, kc, c]
        return np.ascontiguousarray(
            x.reshape(NKC, 128, D).transpose(1, 0, 2)
        ).astype(bf)

    maps = []
    for b in range(B):
        maps.append({
            "query_t": qswizzle(query[b]),
            "key_t": kswizzle(key[b]),
            "value": vswizzle(value[b]),
        })
    return maps


def kernel(query, key, value, mask):
    query = np.asarray(query, dtype=np.float32)
    key = np.asarray(key, dtype=np.float32)
    value = np.asarray(value, dtype=np.float32)
    mask = np.asarray(mask, dtype=np.float32)

    if not np.all(mask == 1.0):
        return _np_reference(query, key, value, mask)

    from concourse.bass_utils import run_bass_kernel_spmd

    if "nc" not in _CACHE:
        _CACHE["nc"] = _build()
    nc = _CACHE["nc"]

    in_maps = make_in_maps(query, key, value)
    last_err = None
    for _ in range(3):  # retry transient device errors (NRT_EXEC_UNIT_...)
        try:
            res = run_bass_kernel_spmd(nc, in_maps, core_ids=list(range(B)))
            break
        except Exception as e:  # noqa: BLE001
            last_err = e
    else:
        raise last_err
    out = np.stack([res.results[b]["out"] for b in range(B)], axis=0)
    return out.astype(np.float32)
